# revision 19
# baseline (speedup 1.0000x reference)
"""Class-align loss (segment_reduce) Trainium2 kernel, v2.

Full inputs: f_source [4,256,128,128] f32, f_convert [4,256,128,128] f32,
seg [4,128,128] int32 (values in [0,19)). Output: scalar f32 triplet loss.

Strategy (data-parallel over batch*h-half, 8 shards; DMA-roofline bound):
  - Each core processes a [256, 8192] shard of each feature tensor.
    Staging DMAs cast fp32 -> bf16 in flight (SWDGE); per-core HBM read
    is 16.8 MB -> ~47 us floor at 358 GB/s.
  - Pixels are processed in batches of 8 groups x 128 pixels (one
    2-bank PSUM tile): PE transposes sixteen [128c,128p] bf16 blocks
    into psumT [128p, 8*256c]; one DVE copy moves it to SBUF.
  - Per-pixel norms: squares are computed on the NATURAL-layout staging
    tiles (lo half on ACT, hi half alternating ACT/DVE), then tiny
    ones-column PE matmuls reduce over channels directly into a
    partition-oriented PSUM tile ss[128p, 8] (2 matmuls per group,
    lo
    +hi accumulate).  This avoids the slow DVE tensor_reduce (1x mode)
    and per-group ACT accumulate ops entirely.
  - w generation is one DVE op per batch: host-precomputed one-hot
    (bf16, in the aux input) times a stride-0 broadcast AP of
    r = 1/||x||: w8[p, g*19+k] = onehot[p,...] * r[p, g].
  - Class sums accumulate with the data-stationary orientation
    (lhsT = xT chunk [128p,128c-half], rhs = w [128p,19]) into four
    [128, 19] f32 PSUM regions (s/c x lo/hi halves), 64 groups each.
  - The acc matmuls run one batch behind the transposes (software
    pipeline) so PE never stalls on the sqrt -> recip -> w chain.
  - Each core writes its [128, 4*19] partial sums; the host sums the 8
    partials and computes the tiny 19-class triplet-loss epilogue in
    float64.

The walrus build encodes at most ONE sync wait per instruction; the
emission order below is arranged so every instruction needs at most one
(absorber transposes take the staging-DMA waits on PE; the vector clock
subsumes all WAR hazards given the buffer depths used).
"""

import sys

import numpy as np

if "/opt/trn_rl_repo" not in sys.path:
    sys.path.insert(0, "/opt/trn_rl_repo")

import ml_dtypes

import concourse.bass as bass
import concourse.mybir as mybir
import concourse.tile as tile
from concourse.bass_utils import run_bass_kernel_spmd
from concourse.tile import add_dep_helper
from concourse.vector_clock import ScopedClock


def _split_drain_and_barrier(self, tick_clock, wait_clock):
    """Tile's kernel-tail drain carries one wait per semaphore the kernel
    ever used; split the excess onto dedicated sequencer NOPs (the 1-wait
    walrus encoding limit)."""
    nc = self.nc
    drain_inst = nc.sync.drain()
    wait_clock.add_sem_waits(
        drain_inst.ins, ScopedClock({None: tick_clock.global_clock})
    )
    si = drain_inst.ins.sync_info
    if si is not None and len(si.on_wait) > 1:
        waits = list(si.on_wait)
        upds = list(si.on_update)
        drain_inst.ins.sync_info = mybir.SyncInfo(
            on_wait=waits[:1], on_update=upds)
        for k in range(1, len(waits)):
            nop = nc.sync.nop(nofuse=True, hint=f"drain_wait_{k}")
            nop.ins.sync_info = mybir.SyncInfo(
                on_wait=[waits[k]], on_update=[])
    nc.all_engine_barrier()
    assert self.sems is not None
    popped = nc._tile_sem_poison_stack.pop()
    assert popped is self._sem_poison
    nc.clear_and_free_semaphores(list(self.sems.allocated().values()))
    nc.all_engine_barrier()


tile.TileContext._drain_and_barrier = _split_drain_and_barrier


def _split_excess_waits(nc):
    """Walrus encodes at most ONE sync wait per instruction; move any
    excess waits onto dedicated same-engine sequencer NOPs spliced in
    immediately before the offending instruction."""
    n = 0
    for f in nc.m.functions:
        for bb in f.blocks:
            out = []
            changed = False
            for ins in bb.instructions:
                si = ins.sync_info
                if si is not None and len(si.on_wait) > 1:
                    waits = list(si.on_wait)
                    for w in waits[:-1]:
                        nop = mybir.InstNoOp(name=f"I-waitsplit-{n}")
                        n += 1
                        nop.engine = ins.engine
                        nop.bass_nofuse = True
                        nop.sync_info = mybir.SyncInfo(on_wait=[w],
                                                       on_update=[])
                        out.append(nop)
                    ins.sync_info = mybir.SyncInfo(
                        on_wait=[waits[-1]], on_update=list(si.on_update))
                    changed = True
                out.append(ins)
            if changed:
                bb.instructions = out

# Problem constants (hardcoded; kernel.py must be self-contained).
B, C, H, W = 4, 256, 128, 128
N_CLASS = 19
N_CORES = 8
EPS_NORM = 1e-12
EPS_TRIP = 1e-6
MARGIN = 0.2

P = 128                      # SBUF partitions / pixel-group size
NPIX = B * H * W // N_CORES  # 8192 pixels per core
NG = NPIX // P               # 64 pixel groups per core
GPB = 8                      # pixel groups per batch (2 PSUM banks)
BPIX = GPB * P               # 1024 pixels per batch
NB = NG // GPB               # 8 batches per tensor
KW = GPB * N_CLASS           # 152 w columns per batch

# aux layout (bf16): identity [0:128) | ones col [128:129) | onehot
AUX_OH = P + 1
AUX_COLS = AUX_OH + NG * N_CLASS   # 129 + 1216 = 1345

_NC_CACHE = {}


def build_nc():
    f32 = mybir.dt.float32
    bf16 = mybir.dt.bfloat16
    Square = mybir.ActivationFunctionType.Square
    nc = bass.Bass(dynamic_dma_scratch_size=32768)

    fs_dram = nc.declare_dram_parameter("f_source", [C, NPIX], f32,
                                        isOutput=False)
    aux_dram = nc.declare_dram_parameter("aux", [P, AUX_COLS], bf16,
                                         isOutput=False)
    fc_dram = nc.declare_dram_parameter("f_convert", [C, NPIX], f32,
                                        isOutput=False)
    out_dram = nc.declare_dram_parameter("out", [P, 4 * N_CLASS], f32,
                                         isOutput=True)
    drams = {"s": fs_dram, "c": fc_dram}

    with tile.TileContext(nc) as tc:
        with (
            tc.tile_pool(name="const", bufs=1) as const_pool,
            tc.tile_pool(name="stage", bufs=1) as stage_pool,
            tc.tile_pool(name="work", bufs=4) as work_pool,
            tc.tile_pool(name="psum_t", bufs=3, space="PSUM") as psum_t_pool,
            tc.tile_pool(name="psum_ss", bufs=1, space="PSUM") as psum_ss_pool,
            tc.tile_pool(name="psum_acc", bufs=1, space="PSUM") as psum_acc_pool,
        ):
            # Constants arrive via one HWDGE DMA (no Q7 involvement).
            aux_sb = const_pool.tile([P, AUX_COLS], bf16, tag="aux")
            nc.sync.dma_start(out=aux_sb[:], in_=aux_dram[:])
            identity = aux_sb[:, 0:P]
            ones_col = aux_sb[:, P:P + 1]

            # Transposed class-sum accumulators. PSUM accumulation groups
            # are bank-granular: regions whose groups are open concurrently
            # must live in different banks (lo vs hi), and the s/c groups
            # within a bank are serialized by the all-s-then-all-c schedule.
            acc_lo = psum_acc_pool.tile([P, 2 * N_CLASS], f32, tag="acc_lo",
                                        name="acc_lo", padded_shape=[P, 512])
            acc_hi = psum_acc_pool.tile([P, 2 * N_CLASS], f32, tag="acc_hi",
                                        name="acc_hi", padded_shape=[P, 512])

            mm_all = []

            def order_after_mm(inst, back):
                if len(mm_all) >= back:
                    add_dep_helper(inst.ins, mm_all[-back].ins, sync=False,
                                   reason="pace Q7 descriptor generation")

            def chain(mm):
                # (Strict emission-order chaining was tried here and lost
                # ~4 us: the scheduler's own grouping pipelines LDWEIGHTS
                # better than a forced O|A|T interleave.)
                mm_all.append(mm)

            # ---- build the global batch schedule -------------------------
            # One staging chunk per batch: the batch's two DMAs complete at
            # consumption cadence, so the pipeline never bubbles waiting for
            # a multi-batch chunk's final byte.
            sched = []   # (t, pix0)
            for t in ("s", "c"):
                for ci in range(NB):
                    sched.append((t, ci * BPIX))
            NBT = len(sched)

            # staging tiles, dedicated per (batch, half)
            stage = {}
            for k, (t, pix0) in enumerate(sched):
                stage[(k, "lo")] = stage_pool.tile(
                    [P, BPIX], bf16, tag=f"st_lo_{k}", name=f"st_lo_{k}")
                stage[(k, "hi")] = stage_pool.tile(
                    [P, BPIX], bf16, tag=f"st_hi_{k}", name=f"st_hi_{k}")

            # Warm-up transposes: take the aux-DMA wait on PE and spin the
            # HAM activity monitor up to 2.4 GHz during the DMA-wait ramp.
            warm = psum_t_pool.tile([P, GPB * P], bf16, tag="pt_bf",
                                    name="warm", bufs=2,
                                    padded_shape=[P, GPB * P])
            for _ in range(14):
                chain(nc.tensor.transpose(warm[:, 0:P], identity, identity))

            group_cnt = {"s": 0, "c": 0}   # global group index per tensor
            tj = {"s": 0, "c": 1}
            issued_chunks = set()
            # Per-batch state for the software pipeline (lag structure:
            # cycle k runs T(k) | O(k-1) | A(k-3) on PE).
            meta = {}    # k -> (t, g0, lo, hi, b0)
            sqlo = {}
            sqhi = {}
            sshandle = {}
            psumB = {}   # lo-half transposed, bf16 (transpose-mode PE)
            psumF = {}   # hi-half transposed, f32 (normal-mode PE)
            xTlo = {}
            xThi = {}
            w8 = {}

            def emit_T(k, g, h):
                m = meta[k]
                px = m[4] + g * P
                if h == 0:
                    # lo half: bf16 transpose-mode (chains at ~116 ns)
                    mm = nc.tensor.transpose(
                        psumB[k][:, g * P:(g + 1) * P],
                        m[2][:, px:px + P], identity)
                else:
                    # hi half: normal-mode matmul against identity -> f32
                    # PSUM; pipelines at ~28 ns (LDWEIGHTS hides).
                    mm = nc.tensor.matmul(
                        psumF[k][g // 4][:, (g % 4) * P:(g % 4 + 1) * P],
                        lhsT=m[3][:, px:px + P], rhs=identity,
                        start=True, stop=True)
                chain(mm)

            def emit_O(k, g, h):
                sq = sqlo[k] if h == 0 else sqhi[k]
                mm = nc.tensor.matmul(
                    sshandle[k][:, g:g + 1], lhsT=sq[:, g * P:(g + 1) * P],
                    rhs=ones_col, start=(h == 0), stop=(h == 1))
                chain(mm)

            def emit_A(k, g, h):
                t, g0 = meta[k][0], meta[k][1]
                G = g0 + g
                acc = acc_lo if h == 0 else acc_hi
                mm = nc.tensor.matmul(
                    acc[:, tj[t] * N_CLASS:(tj[t] + 1) * N_CLASS],
                    lhsT=(xTlo if h == 0 else xThi)[k][:, g * P:(g + 1) * P],
                    rhs=w8[k][:, g * N_CLASS:(g + 1) * N_CLASS],
                    start=(G == 0), stop=(G == NG - 1))
                chain(mm)

            for k in range(NBT + 3):
                hasT = k < NBT
                hasO = 0 <= k - 1 < NBT
                hasA = 0 <= k - 3 < NBT
                chunk_first = False

                if hasT:
                    t, pix0 = sched[k]
                    lo = stage[(k, "lo")]
                    hi = stage[(k, "hi")]
                    chunk_first = True
                    # SWDGE staging DMAs cast fp32 -> bf16 in flight.
                    nc.gpsimd.dma_start(
                        out=lo[:], in_=drams[t][0:P, pix0:pix0 + BPIX])
                    nc.gpsimd.dma_start(
                        out=hi[:], in_=drams[t][P:C, pix0:pix0 + BPIX])
                    g0 = group_cnt[t]
                    group_cnt[t] += GPB
                    meta[k] = (t, g0, lo, hi, 0)
                    psumB[k] = psum_t_pool.tile([P, GPB * P], bf16,
                                                tag="pt_bf", bufs=2,
                                                name=f"psumB_{k}",
                                                padded_shape=[P, GPB * P])
                    psumF[k] = tuple(
                        psum_t_pool.tile([P, GPB * P // 2], f32,
                                         tag="pt_f32", bufs=3,
                                         name=f"psumF_{k}_{w}",
                                         padded_shape=[P, GPB * P // 2])
                        for w in (0, 1))

                # --- PE: interleave O(k-1) | A(k-3) | T(k) ----------------
                # (T last in each sextet: the post-transpose pipeline bubble
                # lands on cheap O/A issues, not on the next transpose.)
                if chunk_first and (hasO or hasA):
                    # Chunk-first cycles: old work first so the PE queue has
                    # useful instructions while the chunk's DMA completes.
                    for g in range(GPB):
                        if hasO:
                            emit_O(k - 1, g, 0)
                            emit_O(k - 1, g, 1)
                        if hasA:
                            emit_A(k - 3, g, 0)
                            emit_A(k - 3, g, 1)
                    for g in range(GPB):
                        emit_T(k, g, 0)
                        emit_T(k, g, 1)
                else:
                    for g in range(GPB):
                        for h in (0, 1):
                            if hasO:
                                emit_O(k - 1, g, h)
                            if hasA:
                                emit_A(k - 3, g, h)
                            if hasT:
                                emit_T(k, g, h)

                # --- ACT: sq_hi (even k), xT-hi cast copy, sqrt of k-1 ----
                if hasT:
                    m = meta[k]
                    sqlo[k] = work_pool.tile([P, BPIX], bf16, tag="sq_lo",
                                             name=f"sqlo_{k}", bufs=2)
                    sqhi[k] = work_pool.tile([P, BPIX], bf16, tag="sq_hi",
                                             name=f"sqhi_{k}", bufs=2)
                    if k % 2 == 0:
                        nc.scalar.activation(sqhi[k][:], m[3][:, 0:BPIX],
                                             Square)
                if hasO:
                    xThi[k - 1] = work_pool.tile([P, GPB * P], bf16,
                                                 tag="xT_hi",
                                                 name=f"xThi_{k - 1}", bufs=4)
                    half = GPB * P // 2
                    for w in (0, 1):
                        nc.scalar.activation(
                            xThi[k - 1][:, w * half:(w + 1) * half],
                            psumF[k - 1][w][:],
                            mybir.ActivationFunctionType.Copy)
                    nrm = work_pool.tile([P, GPB], f32, tag="nrm", bufs=4)
                    nc.scalar.sqrt(nrm[:], sshandle[k - 1][:])
                    meta[k - 1] += (nrm,)

                # --- DVE: recip/w8 of k-2, squares, xT-lo copy of k-1 -----
                if 0 <= k - 2 < NBT:
                    nrm2 = meta[k - 2][5]
                    r = work_pool.tile([P, GPB], f32, tag="r", bufs=2)
                    nc.vector.reciprocal(r[:], nrm2[:])
                    w8[k - 2] = work_pool.tile([P, KW], bf16, tag="w8",
                                               name=f"w8_{k - 2}", bufs=3)
                    g0 = meta[k - 2][1]
                    oh = aux_sb[:, AUX_OH + g0 * N_CLASS:
                                AUX_OH + (g0 + GPB) * N_CLASS]
                    nc.vector.tensor_tensor(
                        out=w8[k - 2][:].rearrange("p (g k) -> p g k", g=GPB),
                        in0=oh.rearrange("p (g k) -> p g k", g=GPB),
                        in1=r[:, 0:GPB].broadcast_to([P, GPB, N_CLASS]),
                        op=mybir.AluOpType.mult)
                if hasT:
                    m = meta[k]
                    nc.vector.tensor_tensor(
                        out=sqlo[k][:], in0=m[2][:, 0:BPIX],
                        in1=m[2][:, 0:BPIX], op=mybir.AluOpType.mult)
                    if k % 2 == 1:
                        nc.vector.tensor_tensor(
                            out=sqhi[k][:], in0=m[3][:, 0:BPIX],
                            in1=m[3][:, 0:BPIX], op=mybir.AluOpType.mult)
                if hasO:
                    xTlo[k - 1] = work_pool.tile([P, GPB * P], bf16,
                                                 tag="xT_lo",
                                                 name=f"xTlo_{k - 1}", bufs=4)
                    nc.vector.tensor_copy(xTlo[k - 1][:], psumB[k - 1][:])

                # ss tile for batch k (written by O(k) next cycle).
                if hasT:
                    sshandle[k] = psum_ss_pool.tile([P, GPB], f32, tag="ss",
                                                    name=f"ss_{k}",
                                                    padded_shape=[P, 512])

            out_sb = work_pool.tile([P, 4 * N_CLASS], f32, tag="out_sb")
            for j, (srcacc, col) in enumerate(
                    ((acc_lo, 0), (acc_hi, 0), (acc_lo, 1), (acc_hi, 1))):
                nc.vector.tensor_copy(
                    out_sb[:, j * N_CLASS:(j + 1) * N_CLASS],
                    srcacc[:, col * N_CLASS:(col + 1) * N_CLASS])
            nc.sync.dma_start(out=out_dram[:], in_=out_sb[:])

    if _NC_CACHE.get("skip_wait_split") is not True:
        _split_excess_waits(nc)
    return nc


def aux_array(seg_flat):
    """Per-core aux input: identity | ones | one-hot(seg), all bf16."""
    ident = np.eye(P, dtype=np.float32)
    ones = np.ones((P, 1), dtype=np.float32)
    segg = seg_flat.reshape(NG, P)                     # [group, p]
    oh = (segg[:, :, None] == np.arange(N_CLASS)[None, None, :])
    oh = oh.astype(np.float32).transpose(1, 0, 2).reshape(P, NG * N_CLASS)
    aux = np.concatenate([ident, ones, oh], axis=1)
    return np.ascontiguousarray(aux.astype(ml_dtypes.bfloat16))


def shard_inputs(f_source, f_convert, seg):
    """Split by (batch, h-half) into 8 per-core input maps."""
    in_maps = []
    hh = H // 2
    for core in range(N_CORES):
        b, half = divmod(core, 2)
        h0 = half * hh
        seg_flat = np.ascontiguousarray(seg[b, h0:h0 + hh, :]).reshape(NPIX)
        in_maps.append({
            "f_source": np.ascontiguousarray(
                f_source[b, :, h0:h0 + hh, :]).reshape(C, NPIX),
            "f_convert": np.ascontiguousarray(
                f_convert[b, :, h0:h0 + hh, :]).reshape(C, NPIX),
            "aux": aux_array(seg_flat),
        })
    return in_maps


def unpack_partial(p):
    """[128, 4*19] per-core partial -> (S, C) each [19, 256]."""
    blocks = [p[:, j * N_CLASS:(j + 1) * N_CLASS] for j in range(4)]
    s = np.concatenate([blocks[0], blocks[1]], axis=0).T
    c = np.concatenate([blocks[2], blocks[3]], axis=0).T
    return s, c


def epilogue(S, Csum):
    """Tiny triplet-loss tail on [19,256] class sums (float64 host math)."""
    n = float(B * H * W)
    cs = S.astype(np.float64) / n
    cc = Csum.astype(np.float64) / n
    cs = cs / np.maximum(np.linalg.norm(cs, axis=1, keepdims=True), EPS_NORM)
    cc = cc / np.maximum(np.linalg.norm(cc, axis=1, keepdims=True), EPS_NORM)
    D = np.linalg.norm(cs[:, None, :] - cc[None, :, :] + EPS_TRIP, axis=2)
    d_ap = np.diag(D)
    terms = np.maximum(d_ap[:, None] - D + MARGIN, 0.0)
    mask = 1.0 - np.eye(N_CLASS)
    loss = (terms * mask).sum() / (N_CLASS * (N_CLASS - 1))
    return np.float32(loss)


def kernel(f_source, f_convert, seg):
    if "nc" not in _NC_CACHE:
        _NC_CACHE["nc"] = build_nc()
    nc = _NC_CACHE["nc"]
    in_maps = shard_inputs(f_source, f_convert, seg)
    res = run_bass_kernel_spmd(nc, in_maps, core_ids=list(range(N_CORES)))
    S = np.zeros((N_CLASS, C), dtype=np.float64)
    Csum = np.zeros((N_CLASS, C), dtype=np.float64)
    for r in res.results:
        s, c = unpack_partial(r["out"].astype(np.float64))
        S += s
        Csum += c
    return epilogue(S, Csum)


if __name__ == "__main__":
    rng = np.random.default_rng(0)
    fs = rng.standard_normal((B, C, H, W), dtype=np.float32)
    fc = rng.standard_normal((B, C, H, W), dtype=np.float32)
    sg = rng.integers(0, N_CLASS, size=(B, H, W), dtype=np.int32)
    print(kernel(fs, fc, sg))


# revision 21
# speedup vs baseline: 1.0459x; 1.0459x over previous
"""Class-align loss (segment_reduce) Trainium2 kernel, v2.

Full inputs: f_source [4,256,128,128] f32, f_convert [4,256,128,128] f32,
seg [4,128,128] int32 (values in [0,19)). Output: scalar f32 triplet loss.

Strategy (data-parallel over batch*h-half, 8 shards; DMA-roofline bound):
  - Each core processes a [256, 8192] shard of each feature tensor.
    Staging DMAs cast fp32 -> bf16 in flight (SWDGE); per-core HBM read
    is 16.8 MB -> ~47 us floor at 358 GB/s.
  - Pixels are processed in batches of 8 groups x 128 pixels (one
    2-bank PSUM tile): PE transposes sixteen [128c,128p] bf16 blocks
    into psumT [128p, 8*256c]; one DVE copy moves it to SBUF.
  - Per-pixel norms: squares are computed on the NATURAL-layout staging
    tiles (lo half on ACT, hi half alternating ACT/DVE), then tiny
    ones-column PE matmuls reduce over channels directly into a
    partition-oriented PSUM tile ss[128p, 8] (2 matmuls per group,
    lo
    +hi accumulate).  This avoids the slow DVE tensor_reduce (1x mode)
    and per-group ACT accumulate ops entirely.
  - w generation is one DVE op per batch: host-precomputed one-hot
    (bf16, in the aux input) times a stride-0 broadcast AP of
    r = 1/||x||: w8[p, g*19+k] = onehot[p,...] * r[p, g].
  - Class sums accumulate with the data-stationary orientation
    (lhsT = xT chunk [128p,128c-half], rhs = w [128p,19]) into four
    [128, 19] f32 PSUM regions (s/c x lo/hi halves), 64 groups each.
  - The acc matmuls run one batch behind the transposes (software
    pipeline) so PE never stalls on the sqrt -> recip -> w chain.
  - Each core writes its [128, 4*19] partial sums; the host sums the 8
    partials and computes the tiny 19-class triplet-loss epilogue in
    float64.

The walrus build encodes at most ONE sync wait per instruction; the
emission order below is arranged so every instruction needs at most one
(absorber transposes take the staging-DMA waits on PE; the vector clock
subsumes all WAR hazards given the buffer depths used).
"""

import sys

import numpy as np

if "/opt/trn_rl_repo" not in sys.path:
    sys.path.insert(0, "/opt/trn_rl_repo")

import ml_dtypes

import concourse.bass as bass
import concourse.mybir as mybir
import concourse.tile as tile
from concourse.bass_utils import run_bass_kernel_spmd
from concourse.tile import add_dep_helper
from concourse.vector_clock import ScopedClock


def _split_drain_and_barrier(self, tick_clock, wait_clock):
    """Tile's kernel-tail drain carries one wait per semaphore the kernel
    ever used; split the excess onto dedicated sequencer NOPs (the 1-wait
    walrus encoding limit)."""
    nc = self.nc
    drain_inst = nc.sync.drain()
    wait_clock.add_sem_waits(
        drain_inst.ins, ScopedClock({None: tick_clock.global_clock})
    )
    si = drain_inst.ins.sync_info
    if si is not None and len(si.on_wait) > 1:
        waits = list(si.on_wait)
        upds = list(si.on_update)
        drain_inst.ins.sync_info = mybir.SyncInfo(
            on_wait=waits[:1], on_update=upds)
        for k in range(1, len(waits)):
            nop = nc.sync.nop(nofuse=True, hint=f"drain_wait_{k}")
            nop.ins.sync_info = mybir.SyncInfo(
                on_wait=[waits[k]], on_update=[])
    nc.all_engine_barrier()
    assert self.sems is not None
    popped = nc._tile_sem_poison_stack.pop()
    assert popped is self._sem_poison
    nc.clear_and_free_semaphores(list(self.sems.allocated().values()))
    nc.all_engine_barrier()


tile.TileContext._drain_and_barrier = _split_drain_and_barrier


def _split_excess_waits(nc):
    """Walrus encodes at most ONE sync wait per instruction; move any
    excess waits onto dedicated same-engine sequencer NOPs spliced in
    immediately before the offending instruction."""
    n = 0
    for f in nc.m.functions:
        for bb in f.blocks:
            out = []
            changed = False
            for ins in bb.instructions:
                si = ins.sync_info
                if si is not None and len(si.on_wait) > 1:
                    waits = list(si.on_wait)
                    for w in waits[:-1]:
                        nop = mybir.InstNoOp(name=f"I-waitsplit-{n}")
                        n += 1
                        nop.engine = ins.engine
                        nop.bass_nofuse = True
                        nop.sync_info = mybir.SyncInfo(on_wait=[w],
                                                       on_update=[])
                        out.append(nop)
                    ins.sync_info = mybir.SyncInfo(
                        on_wait=[waits[-1]], on_update=list(si.on_update))
                    changed = True
                out.append(ins)
            if changed:
                bb.instructions = out

# Problem constants (hardcoded; kernel.py must be self-contained).
B, C, H, W = 4, 256, 128, 128
N_CLASS = 19
N_CORES = 8
EPS_NORM = 1e-12
EPS_TRIP = 1e-6
MARGIN = 0.2

P = 128                      # SBUF partitions / pixel-group size
NPIX = B * H * W // N_CORES  # 8192 pixels per core
NG = NPIX // P               # 64 pixel groups per core
GPB = 8                      # pixel groups per batch (2 PSUM banks)
BPIX = GPB * P               # 1024 pixels per batch
NB = NG // GPB               # 8 batches per tensor
KW = GPB * N_CLASS           # 152 w columns per batch

# aux layout (bf16): identity [0:128) | ones col [128:129) | onehot
AUX_OH = P + 1
AUX_COLS = AUX_OH + NG * N_CLASS   # 129 + 1216 = 1345

_NC_CACHE = {}


def build_nc():
    f32 = mybir.dt.float32
    bf16 = mybir.dt.bfloat16
    Square = mybir.ActivationFunctionType.Square
    nc = bass.Bass(dynamic_dma_scratch_size=32768)

    fs_dram = nc.declare_dram_parameter("f_source", [C, NPIX], f32,
                                        isOutput=False)
    aux_dram = nc.declare_dram_parameter("aux", [P, AUX_COLS], bf16,
                                         isOutput=False)
    fc_dram = nc.declare_dram_parameter("f_convert", [C, NPIX], f32,
                                        isOutput=False)
    out_dram = nc.declare_dram_parameter("out", [P, 4 * N_CLASS], f32,
                                         isOutput=True)
    drams = {"s": fs_dram, "c": fc_dram}

    with tile.TileContext(nc) as tc:
        with (
            tc.tile_pool(name="const", bufs=1) as const_pool,
            tc.tile_pool(name="stage", bufs=1) as stage_pool,
            tc.tile_pool(name="work", bufs=4) as work_pool,
            tc.tile_pool(name="psum_t", bufs=3, space="PSUM") as psum_t_pool,
            tc.tile_pool(name="psum_ss", bufs=1, space="PSUM") as psum_ss_pool,
            tc.tile_pool(name="psum_acc", bufs=1, space="PSUM") as psum_acc_pool,
        ):
            # Constants arrive via one HWDGE DMA (no Q7 involvement).
            aux_sb = const_pool.tile([P, AUX_COLS], bf16, tag="aux")
            nc.sync.dma_start(out=aux_sb[:], in_=aux_dram[:])
            identity = aux_sb[:, 0:P]
            ones_col = aux_sb[:, P:P + 1]

            # Transposed class-sum accumulators. PSUM accumulation groups
            # are bank-granular: regions whose groups are open concurrently
            # must live in different banks (lo vs hi), and the s/c groups
            # within a bank are serialized by the all-s-then-all-c schedule.
            acc_lo = psum_acc_pool.tile([P, 2 * N_CLASS], f32, tag="acc_lo",
                                        name="acc_lo", padded_shape=[P, 512])
            acc_hi = psum_acc_pool.tile([P, 2 * N_CLASS], f32, tag="acc_hi",
                                        name="acc_hi", padded_shape=[P, 512])

            mm_all = []

            def order_after_mm(inst, back):
                if len(mm_all) >= back:
                    add_dep_helper(inst.ins, mm_all[-back].ins, sync=False,
                                   reason="pace Q7 descriptor generation")

            def chain(mm):
                # (Strict emission-order chaining was tried here and lost
                # ~4 us: the scheduler's own grouping pipelines LDWEIGHTS
                # better than a forced O|A|T interleave.)
                mm_all.append(mm)

            # ---- build the global batch schedule -------------------------
            # One staging chunk per batch: the batch's two DMAs complete at
            # consumption cadence, so the pipeline never bubbles waiting for
            # a multi-batch chunk's final byte.
            sched = []   # (t, pix0)
            for t in ("s", "c"):
                for ci in range(NB):
                    sched.append((t, ci * BPIX))
            NBT = len(sched)

            # staging tiles, dedicated per (batch, half)
            stage = {}
            for k, (t, pix0) in enumerate(sched):
                stage[(k, "lo")] = stage_pool.tile(
                    [P, BPIX], bf16, tag=f"st_lo_{k}", name=f"st_lo_{k}")
                stage[(k, "hi")] = stage_pool.tile(
                    [P, BPIX], bf16, tag=f"st_hi_{k}", name=f"st_hi_{k}")

            # Warm-up transposes: take the aux-DMA wait on PE and spin the
            # HAM activity monitor up to 2.4 GHz during the DMA-wait ramp.
            warm = psum_t_pool.tile([P, GPB * C], bf16, tag="pt",
                                    name="warm", bufs=2,
                                    padded_shape=[P, GPB * C])
            for _ in range(14):
                chain(nc.tensor.transpose(warm[:, 0:P], identity, identity))

            group_cnt = {"s": 0, "c": 0}   # global group index per tensor
            tj = {"s": 0, "c": 1}
            issued_chunks = set()
            # Per-batch state for the software pipeline (lag structure:
            # cycle k runs T(k) | O(k-1) | A(k-3) on PE).
            meta = {}    # k -> (t, g0, lo, hi, b0)
            sqlo = {}
            sqhi = {}
            sshandle = {}
            psumT = {}
            xT = {}
            w8 = {}

            def emit_T(k, g, h):
                m = meta[k]
                px = m[4] + g * P
                mm = nc.tensor.transpose(
                    psumT[k][:, g * C + h * P:g * C + (h + 1) * P],
                    (m[2] if h == 0 else m[3])[:, px:px + P], identity)
                chain(mm)

            def emit_O(k, g, h):
                sq = sqlo[k] if h == 0 else sqhi[k]
                mm = nc.tensor.matmul(
                    sshandle[k][:, g:g + 1], lhsT=sq[:, g * P:(g + 1) * P],
                    rhs=ones_col, start=(h == 0), stop=(h == 1))
                chain(mm)

            def emit_A(k, g, h):
                t, g0 = meta[k][0], meta[k][1]
                G = g0 + g
                acc = acc_lo if h == 0 else acc_hi
                mm = nc.tensor.matmul(
                    acc[:, tj[t] * N_CLASS:(tj[t] + 1) * N_CLASS],
                    lhsT=xT[k][:, g * C + h * P:g * C + (h + 1) * P],
                    rhs=w8[k][:, g * N_CLASS:(g + 1) * N_CLASS],
                    start=(G == 0), stop=(G == NG - 1))
                chain(mm)

            for k in range(NBT + 3):
                hasT = k < NBT
                hasO = 0 <= k - 1 < NBT
                hasA = 0 <= k - 3 < NBT
                chunk_first = False

                if hasT:
                    t, pix0 = sched[k]
                    lo = stage[(k, "lo")]
                    hi = stage[(k, "hi")]
                    chunk_first = True
                    # SWDGE staging DMAs cast fp32 -> bf16 in flight.
                    nc.gpsimd.dma_start(
                        out=lo[:], in_=drams[t][0:P, pix0:pix0 + BPIX])
                    nc.gpsimd.dma_start(
                        out=hi[:], in_=drams[t][P:C, pix0:pix0 + BPIX])
                    g0 = group_cnt[t]
                    group_cnt[t] += GPB
                    meta[k] = (t, g0, lo, hi, 0)
                    psumT[k] = psum_t_pool.tile([P, GPB * C], bf16,
                                                tag="pt", bufs=2,
                                                name=f"psumT_{k}",
                                                padded_shape=[P, GPB * C])

                # --- PE: interleave O(k-1) | A(k-3) | T(k) ----------------
                # (T last in each sextet: the post-transpose pipeline bubble
                # lands on cheap O/A issues, not on the next transpose.)
                if chunk_first and (hasO or hasA):
                    # Chunk-first cycles: old work first so the PE queue has
                    # useful instructions while the chunk's DMA completes.
                    for g in range(GPB):
                        if hasO:
                            emit_O(k - 1, g, 0)
                            emit_O(k - 1, g, 1)
                        if hasA:
                            emit_A(k - 3, g, 0)
                            emit_A(k - 3, g, 1)
                    for g in range(GPB):
                        emit_T(k, g, 0)
                        emit_T(k, g, 1)
                else:
                    for g in range(GPB):
                        for h in (0, 1):
                            if hasO:
                                emit_O(k - 1, g, h)
                            if hasA:
                                emit_A(k - 3, g, h)
                            if hasT:
                                emit_T(k, g, h)

                # --- ACT: squares of batch k, sqrt of k-1 (last) ----------
                if hasT:
                    m = meta[k]
                    sqlo[k] = work_pool.tile([P, BPIX], bf16, tag="sq_lo",
                                             name=f"sqlo_{k}", bufs=2)
                    nc.scalar.activation(sqlo[k][:], m[2][:, 0:BPIX], Square)
                    sqhi[k] = work_pool.tile([P, BPIX], bf16, tag="sq_hi",
                                             name=f"sqhi_{k}", bufs=2)
                    if k % 2 == 0:
                        nc.scalar.activation(sqhi[k][:], m[3][:, 0:BPIX],
                                             Square)
                if hasO:
                    nrm = work_pool.tile([P, GPB], f32, tag="nrm", bufs=4)
                    nc.scalar.sqrt(nrm[:], sshandle[k - 1][:])
                    meta[k - 1] += (nrm,)

                # --- DVE: recip/w8 of k-2, squares, xT-lo copy of k-1 -----
                if 0 <= k - 2 < NBT:
                    nrm2 = meta[k - 2][5]
                    r = work_pool.tile([P, GPB], f32, tag="r", bufs=2)
                    nc.vector.reciprocal(r[:], nrm2[:])
                    w8[k - 2] = work_pool.tile([P, KW], bf16, tag="w8",
                                               name=f"w8_{k - 2}", bufs=3)
                    g0 = meta[k - 2][1]
                    oh = aux_sb[:, AUX_OH + g0 * N_CLASS:
                                AUX_OH + (g0 + GPB) * N_CLASS]
                    nc.vector.tensor_tensor(
                        out=w8[k - 2][:].rearrange("p (g k) -> p g k", g=GPB),
                        in0=oh.rearrange("p (g k) -> p g k", g=GPB),
                        in1=r[:, 0:GPB].broadcast_to([P, GPB, N_CLASS]),
                        op=mybir.AluOpType.mult)
                if hasT and k % 2 == 1:
                    m = meta[k]
                    nc.vector.tensor_tensor(
                        out=sqhi[k][:], in0=m[3][:, 0:BPIX],
                        in1=m[3][:, 0:BPIX], op=mybir.AluOpType.mult)
                if hasO:
                    xT[k - 1] = work_pool.tile([P, GPB * C], bf16, tag="xT",
                                               name=f"xT_{k - 1}", bufs=3)
                    nc.vector.tensor_copy(xT[k - 1][:], psumT[k - 1][:])

                # ss tile for batch k (written by O(k) next cycle).
                if hasT:
                    sshandle[k] = psum_ss_pool.tile([P, GPB], f32, tag="ss",
                                                    name=f"ss_{k}", bufs=2,
                                                    padded_shape=[P, 512])

            out_sb = work_pool.tile([P, 4 * N_CLASS], f32, tag="out_sb")
            for j, (srcacc, col) in enumerate(
                    ((acc_lo, 0), (acc_hi, 0), (acc_lo, 1), (acc_hi, 1))):
                nc.vector.tensor_copy(
                    out_sb[:, j * N_CLASS:(j + 1) * N_CLASS],
                    srcacc[:, col * N_CLASS:(col + 1) * N_CLASS])
            nc.sync.dma_start(out=out_dram[:], in_=out_sb[:])

    if _NC_CACHE.get("skip_wait_split") is not True:
        _split_excess_waits(nc)
    return nc


def aux_array(seg_flat):
    """Per-core aux input: identity | ones | one-hot(seg), all bf16."""
    ident = np.eye(P, dtype=np.float32)
    ones = np.ones((P, 1), dtype=np.float32)
    segg = seg_flat.reshape(NG, P)                     # [group, p]
    oh = (segg[:, :, None] == np.arange(N_CLASS)[None, None, :])
    oh = oh.astype(np.float32).transpose(1, 0, 2).reshape(P, NG * N_CLASS)
    aux = np.concatenate([ident, ones, oh], axis=1)
    return np.ascontiguousarray(aux.astype(ml_dtypes.bfloat16))


def shard_inputs(f_source, f_convert, seg):
    """Split by (batch, h-half) into 8 per-core input maps."""
    in_maps = []
    hh = H // 2
    for core in range(N_CORES):
        b, half = divmod(core, 2)
        h0 = half * hh
        seg_flat = np.ascontiguousarray(seg[b, h0:h0 + hh, :]).reshape(NPIX)
        in_maps.append({
            "f_source": np.ascontiguousarray(
                f_source[b, :, h0:h0 + hh, :]).reshape(C, NPIX),
            "f_convert": np.ascontiguousarray(
                f_convert[b, :, h0:h0 + hh, :]).reshape(C, NPIX),
            "aux": aux_array(seg_flat),
        })
    return in_maps


def unpack_partial(p):
    """[128, 4*19] per-core partial -> (S, C) each [19, 256]."""
    blocks = [p[:, j * N_CLASS:(j + 1) * N_CLASS] for j in range(4)]
    s = np.concatenate([blocks[0], blocks[1]], axis=0).T
    c = np.concatenate([blocks[2], blocks[3]], axis=0).T
    return s, c


def epilogue(S, Csum):
    """Tiny triplet-loss tail on [19,256] class sums (float64 host math)."""
    n = float(B * H * W)
    cs = S.astype(np.float64) / n
    cc = Csum.astype(np.float64) / n
    cs = cs / np.maximum(np.linalg.norm(cs, axis=1, keepdims=True), EPS_NORM)
    cc = cc / np.maximum(np.linalg.norm(cc, axis=1, keepdims=True), EPS_NORM)
    D = np.linalg.norm(cs[:, None, :] - cc[None, :, :] + EPS_TRIP, axis=2)
    d_ap = np.diag(D)
    terms = np.maximum(d_ap[:, None] - D + MARGIN, 0.0)
    mask = 1.0 - np.eye(N_CLASS)
    loss = (terms * mask).sum() / (N_CLASS * (N_CLASS - 1))
    return np.float32(loss)


def kernel(f_source, f_convert, seg):
    if "nc" not in _NC_CACHE:
        _NC_CACHE["nc"] = build_nc()
    nc = _NC_CACHE["nc"]
    in_maps = shard_inputs(f_source, f_convert, seg)
    res = run_bass_kernel_spmd(nc, in_maps, core_ids=list(range(N_CORES)))
    S = np.zeros((N_CLASS, C), dtype=np.float64)
    Csum = np.zeros((N_CLASS, C), dtype=np.float64)
    for r in res.results:
        s, c = unpack_partial(r["out"].astype(np.float64))
        S += s
        Csum += c
    return epilogue(S, Csum)


if __name__ == "__main__":
    rng = np.random.default_rng(0)
    fs = rng.standard_normal((B, C, H, W), dtype=np.float32)
    fc = rng.standard_normal((B, C, H, W), dtype=np.float32)
    sg = rng.integers(0, N_CLASS, size=(B, H, W), dtype=np.int32)
    print(kernel(fs, fc, sg))
